# revision 1
# baseline (speedup 1.0000x reference)
"""TRN2 Bass kernel for nn_MultiHeadAttention_78056735637728.

8-way data parallel over batch (B=8, one batch element per NeuronCore).
Host side: the padding mask m (shared across batch/heads/queries) is applied
EXACTLY by gathering only the unmasked kv positions — masked positions
contribute exp(logit - 1e9) == 0.0 in fp32 to every softmax, so dropping
them is bit-equivalent; k/v are gathered and zero-padded to a multiple of
128 and a per-position bias of -1e9 kills the pad rows inside the fused
exp on device.

Device kernel (per core), activations kept feature-major (transposed):
  - Q.T/K.T projections with the weight blocks stationary; V projected to
    natural (seq-major) layout with the input block stationary.
  - logits.T (kv j on partitions, queries i free) via row-packed K=32
    matmuls: head pair (2h, 2h+1) lives at array rows 32*(h%4).
  - P = exp(logits/sqrt(32) + kvbias[j]) fused on ScalarE straight from
    PSUM (this is the bottleneck engine: ~1 elem/lane/cycle).
  - P@V and the softmax denominator share col-packed matmuls into one PSUM
    tile: partitions [num_h0 | num_h1 | den_h0 | den_h1] (denominator via
    an all-ones 32-column stationary).
  - normalize on VectorE: reciprocal_approx_fast(den) * num -> O.T, all
    access patterns at base partition 0 (HW silently drops partition-
    shifted DVE writes).
  - output projection from four 64-row O.T tiles (K=64, one row group).
"""
import math
import sys
from contextlib import ExitStack

import numpy as np

if "/opt/trn_rl_repo" not in sys.path:
    sys.path.insert(0, "/opt/trn_rl_repo")

import concourse.bass as bass  # noqa: E402
import concourse.tile as tile  # noqa: E402
from concourse import bacc, bass_utils, mybir  # noqa: E402
from concourse._compat import with_exitstack  # noqa: E402

F32 = mybir.dt.float32
B = 8
S = 2048
D = 256
H = 8
DH = 32
PDIM = 128
N_CORES = 8

IN_NAMES = ["qt", "kt", "vt", "wq", "wk", "wv", "wo",
            "bq", "bk", "bo", "bvb", "kvb"]


@with_exitstack
def _mha_kernel(ctx: ExitStack, tc: tile.TileContext, outs, ins, SKV):
    nc = tc.nc
    (i_qt, i_kt, i_vt, i_wq, i_wk, i_wv, i_wo,
     i_bq, i_bk, i_bo, i_bvb, i_kvb) = ins
    o_ot = outs[0]

    NJ = SKV // PDIM          # kv tiles of 128
    NQC = S // 512            # query chunks of 512
    assert S % 512 == 0 and SKV % PDIM == 0
    scale = 1.0 / math.sqrt(DH)

    def chunks(total, step=512):
        out = []
        c = 0
        while c < total:
            w = min(step, total - c)
            out.append((c, w))
            c += w
        return out

    consts = ctx.enter_context(tc.tile_pool(name="consts", bufs=1))

    def load2(ap_dram, cols):
        ts = []
        for b in range(2):
            t = consts.tile([PDIM, cols], F32, name=f"{ap_dram.name}_sb{b}")
            nc.sync.dma_start(t[:], ap_dram[b * PDIM:(b + 1) * PDIM, :])
            ts.append(t)
        return ts

    wq_sb = load2(i_wq, D)
    wk_sb = load2(i_wk, D)
    wv_sb = load2(i_wv, D)

    def load_bias(ap_dram, name):
        t = consts.tile([PDIM, 2], F32, name=name)
        nc.sync.dma_start(t[:], ap_dram.rearrange("(b p) -> p b", p=PDIM))
        return t

    bq_sb = load_bias(i_bq, "bq_sb")
    bk_sb = load_bias(i_bk, "bk_sb")
    bo_sb = load_bias(i_bo, "bo_sb")
    bvb_sb = consts.tile([PDIM, D], F32)
    nc.sync.dma_start(bvb_sb[:], i_bvb[:])
    kvb_sb = consts.tile([PDIM, NJ], F32)
    nc.sync.dma_start(kvb_sb[:], i_kvb.rearrange("(j p) -> p j", p=PDIM))
    ones_sb = consts.tile([PDIM, DH], F32)
    nc.vector.memset(ones_sb[:], 1.0)

    acts = ctx.enter_context(tc.tile_pool(name="acts", bufs=1))
    QT = [acts.tile([PDIM, S], F32, name=f"QT{b}") for b in range(2)]
    KT = [acts.tile([PDIM, SKV], F32, name=f"KT{b}") for b in range(2)]
    VN = [acts.tile([PDIM, D], F32, name=f"VN{j}") for j in range(NJ)]
    # O.T as four 64-row tiles so every normalize AP is at base partition 0
    OT = [acts.tile([64, S], F32, name=f"OT{p}") for p in range(4)]
    wo4 = [acts.tile([64, D], F32, name=f"wo4_{p}") for p in range(4)]
    for p in range(4):
        nc.sync.dma_start(wo4[p][:], i_wo[p * 64:(p + 1) * 64, :])

    # ---- projections ----
    with tc.tile_pool(name="proj_in", bufs=1) as proj_in, \
         tc.tile_pool(name="proj_ps", bufs=2, space="PSUM") as proj_ps:
        qt_sb = load2(i_qt, S)
        kt_sb = load2(i_kt, SKV)
        vt_sb = load2(i_vt, SKV)

        for dst, w_sb, b_sb, x_sb, ncols in (
            (QT, wq_sb, bq_sb, qt_sb, S),
            (KT, wk_sb, bk_sb, kt_sb, SKV),
        ):
            for ob in range(2):
                for c0, w in chunks(ncols):
                    ps = proj_ps.tile([PDIM, 512], F32, tag="projps")
                    for ib in range(2):
                        nc.tensor.matmul(
                            ps[:, 0:w],
                            lhsT=w_sb[ib][:, ob * PDIM:(ob + 1) * PDIM],
                            rhs=x_sb[ib][:, c0:c0 + w],
                            start=(ib == 0), stop=(ib == 1),
                        )
                    nc.vector.tensor_scalar_add(
                        dst[ob][:, c0:c0 + w], ps[:, 0:w], b_sb[:, ob:ob + 1])

        for j in range(NJ):
            ps = proj_ps.tile([PDIM, D], F32, tag="projps")
            for ib in range(2):
                nc.tensor.matmul(
                    ps[:],
                    lhsT=vt_sb[ib][:, j * PDIM:(j + 1) * PDIM],
                    rhs=wv_sb[ib][:],
                    start=(ib == 0), stop=(ib == 1),
                )
            nc.vector.tensor_add(VN[j][:], ps[:], bvb_sb[:])

    # ---- attention ----
    with tc.tile_pool(name="lps", bufs=2, space="PSUM") as lps_pool, \
         tc.tile_pool(name="pvps", bufs=2, space="PSUM") as pv_pool, \
         tc.tile_pool(name="psb", bufs=3) as p_pool, \
         tc.tile_pool(name="norm", bufs=2) as norm_pool:
        for ic in range(NQC):
            i0 = ic * 512
            for hp in range(4):
                h0, h1 = 2 * hp, 2 * hp + 1
                pv = pv_pool.tile([PDIM, 512], F32, tag="pv")
                for j in range(NJ):
                    lt = lps_pool.tile([PDIM, 1024], F32, tag="lt")
                    for hh, h in enumerate((h0, h1)):
                        t = h // 4
                        bp = 32 * (h % 4)
                        nc.tensor.matmul(
                            lt[:, hh * 512:(hh + 1) * 512],
                            lhsT=KT[t][bp:bp + 32, j * PDIM:(j + 1) * PDIM],
                            rhs=QT[t][bp:bp + 32, i0:i0 + 512],
                            start=True, stop=True,
                            tile_position=(bp, 0),
                        )
                    pt = p_pool.tile([PDIM, 1024], F32, tag="pt")
                    nc.scalar.activation(
                        pt[:], lt[:], mybir.ActivationFunctionType.Exp,
                        bias=kvb_sb[:, j:j + 1], scale=scale)
                    # col-packed PV: [num_h0 | num_h1 | den_h0 | den_h1]
                    for c, (lhsT, rhs) in enumerate((
                        (VN[j][:, h0 * DH:(h0 + 1) * DH], pt[:, 0:512]),
                        (VN[j][:, h1 * DH:(h1 + 1) * DH], pt[:, 512:1024]),
                        (ones_sb[:], pt[:, 0:512]),
                        (ones_sb[:], pt[:, 512:1024]),
                    )):
                        nc.tensor.matmul(
                            pv[32 * c:32 * (c + 1), :], lhsT=lhsT, rhs=rhs,
                            start=(j == 0), stop=(j == NJ - 1),
                            tile_position=(0, 32 * c),
                            skip_group_check=True,
                        )
                den = norm_pool.tile([64, 512], F32, tag="den")
                nc.vector.tensor_copy(den[:], pv[64:128, :])
                rec = norm_pool.tile([64, 512], F32, tag="rec")
                nc.vector.reciprocal_approx_fast(rec[:], den[:])
                nc.vector.tensor_mul(
                    OT[hp][:, i0:i0 + 512], pv[0:64, :], rec[:])

    # ---- output projection ----
    with tc.tile_pool(name="ops", bufs=2, space="PSUM") as out_ps, \
         tc.tile_pool(name="osb", bufs=2) as out_sb:
        for ob in range(2):
            for c in range(NQC):
                ps = out_ps.tile([PDIM, 512], F32, tag="ops")
                for p in range(4):
                    nc.tensor.matmul(
                        ps[:],
                        lhsT=wo4[p][:, ob * PDIM:(ob + 1) * PDIM],
                        rhs=OT[p][:, c * 512:(c + 1) * 512],
                        start=(p == 0), stop=(p == 3),
                    )
                ft = out_sb.tile([PDIM, 512], F32, tag="ft")
                nc.vector.tensor_scalar_add(ft[:], ps[:], bo_sb[:, ob:ob + 1])
                nc.sync.dma_start(
                    o_ot[ob * PDIM:(ob + 1) * PDIM, c * 512:(c + 1) * 512],
                    ft[:])


_PROGRAM_CACHE = {}


def _get_program(SKV):
    if SKV in _PROGRAM_CACHE:
        return _PROGRAM_CACHE[SKV]
    nc = bacc.Bacc("TRN2", target_bir_lowering=False, debug=False,
                   enable_asserts=False, num_devices=1)
    shapes = dict(qt=(D, S), kt=(D, SKV), vt=(D, SKV), wq=(D, D), wk=(D, D),
                  wv=(D, D), wo=(D, D), bq=(D,), bk=(D,), bo=(D,),
                  bvb=(PDIM, D), kvb=(SKV,))
    in_aps = [nc.dram_tensor(k, shapes[k], F32, kind="ExternalInput").ap()
              for k in IN_NAMES]
    out_ap = nc.dram_tensor("ot", (D, S), F32, kind="ExternalOutput").ap()
    with tile.TileContext(nc) as tc:
        _mha_kernel(tc, [out_ap], in_aps, SKV=SKV)
    nc.compile()
    _PROGRAM_CACHE[SKV] = nc
    return nc


def _prepare_in_maps(q, k, v, m, wq, bq, wk, bk, wv, bv, wo, bo):
    mask = np.asarray(m, np.float32).reshape(-1)
    keep = np.flatnonzero(mask == 0.0)
    skv = len(keep)
    assert skv > 0, "all kv positions masked"
    SKV = max(PDIM, ((skv + PDIM - 1) // PDIM) * PDIM)

    kvb = np.zeros(SKV, np.float32)
    kvb[skv:] = -1e9
    bvb = np.ascontiguousarray(np.tile(np.asarray(bv, np.float32)[None, :],
                                       (PDIM, 1)))
    common = dict(
        wq=np.ascontiguousarray(wq, np.float32),
        wk=np.ascontiguousarray(wk, np.float32),
        wv=np.ascontiguousarray(wv, np.float32),
        wo=np.ascontiguousarray(wo, np.float32),
        bq=np.ascontiguousarray(bq, np.float32),
        bk=np.ascontiguousarray(bk, np.float32),
        bo=np.ascontiguousarray(bo, np.float32),
        bvb=bvb, kvb=kvb,
    )
    in_maps = []
    for b in range(B):
        kg = np.zeros((D, SKV), np.float32)
        vg = np.zeros((D, SKV), np.float32)
        kg[:, :skv] = np.asarray(k[b], np.float32).T[:, keep]
        vg[:, :skv] = np.asarray(v[b], np.float32).T[:, keep]
        in_maps.append(dict(
            qt=np.ascontiguousarray(np.asarray(q[b], np.float32).T),
            kt=kg, vt=vg, **common))
    return in_maps, SKV


def _run(q, k, v, m, wq, bq, wk, bk, wv, bv, wo, bo, trace=False):
    in_maps, SKV = _prepare_in_maps(q, k, v, m, wq, bq, wk, bk, wv, bv, wo, bo)
    nc = _get_program(SKV)
    res = bass_utils.run_bass_kernel_spmd(
        nc, in_maps, core_ids=list(range(N_CORES)), trace=trace)
    out = np.stack([res.results[b]["ot"].T for b in range(B)], axis=0)
    return np.ascontiguousarray(out, np.float32), res


def kernel(q, k, v, m, wq, bq, wk, bk, wv, bv, wo, bo):
    out, _ = _run(q, k, v, m, wq, bq, wk, bk, wv, bv, wo, bo, trace=False)
    return out


# revision 4
# speedup vs baseline: 1.0430x; 1.0430x over previous
"""TRN2 Bass kernel for nn_MultiHeadAttention_78056735637728.

8-way data parallel over batch (B=8, one batch element per NeuronCore).
Host side: the padding mask m (shared across batch/heads/queries) is applied
EXACTLY by gathering only the unmasked kv positions — masked positions
contribute exp(logit - 1e9) == 0.0 in fp32 to every softmax, so dropping
them is bit-equivalent; k/v are gathered and zero-padded to a multiple of
128 and a per-position bias of -1e9 kills the pad rows inside the fused
exp on device.

Device kernel (per core), activations kept feature-major (transposed), all
matmul operands in float32r (TF32-class, 4x faster than fp32 on the PE):
  - Q.T/K.T projections with the weight blocks stationary; V projected into
    an interleaved [ones | V_h0 | V_h1 | ones] layout per kv tile.
  - logits.T (kv j on partitions, queries i free) via row-packed K=32
    matmuls: head pair (2hp, 2hp+1) at array rows 32*(h%4).
  - P = exp(logits/sqrt(32) + kvbias[j]) fused on ScalarE straight from
    PSUM (bottleneck engine: 1 elem/lane/cycle).
  - P@V and the softmax denominator fused per head into ONE 64-column
    stationary matmul: out rows [den_h0 | num_h0] (col group 0) and
    [num_h1 | den_h1] (col group 64) accumulated over kv tiles.
  - normalize on VectorE: gather dens -> reciprocal_approx_fast -> one
    tensor_mul over both heads' nums; every SBUF AP at base partition 0
    (HW silently drops partition-shifted DVE writes between SBUF APs).
  - output projection from four 64-row O.T tiles (K=64, one row group).
"""
import math
import sys
from contextlib import ExitStack

import numpy as np

if "/opt/trn_rl_repo" not in sys.path:
    sys.path.insert(0, "/opt/trn_rl_repo")

import concourse.bass as bass  # noqa: E402
import concourse.tile as tile  # noqa: E402
from concourse import bacc, bass_utils, mybir  # noqa: E402
from concourse._compat import with_exitstack  # noqa: E402

F32 = mybir.dt.float32
F32R = mybir.dt.float32r
B = 8
S = 2048
D = 256
H = 8
DH = 32
PDIM = 128
N_CORES = 8

IN_NAMES = ["qt", "kt", "vt", "wq", "wk", "wv", "wo",
            "bq", "bk", "bo", "bvb", "kvb", "vne1"]


@with_exitstack
def _mha_kernel(ctx: ExitStack, tc: tile.TileContext, outs, ins, SKV, S=S):
    nc = tc.nc
    (i_qt, i_kt, i_vt, i_wq, i_wk, i_wv, i_wo,
     i_bq, i_bk, i_bo, i_bvb, i_kvb, i_vne1) = ins
    o_ot = outs[0]

    NJ = SKV // PDIM          # kv tiles of 128
    NQC = S // 512            # query chunks of 512
    assert S % 512 == 0 and SKV % PDIM == 0
    scale = 1.0 / math.sqrt(DH)

    def chunks(total, step=512):
        out = []
        c = 0
        while c < total:
            w = min(step, total - c)
            out.append((c, w))
            c += w
        return out

    consts = ctx.enter_context(tc.tile_pool(name="consts", bufs=1))

    # ---- load weights / biases / inputs (DRAM side already float32r) ----
    def load2(ap_dram, cols):
        ts = []
        for b in range(2):
            t = consts.tile([PDIM, cols], F32R, name=f"{ap_dram.name}_sb{b}")
            nc.sync.dma_start(t[:], ap_dram[b * PDIM:(b + 1) * PDIM, :])
            ts.append(t)
        return ts

    wq_sb = load2(i_wq, D)
    wk_sb = load2(i_wk, D)
    wv_sb = load2(i_wv, D)

    def load_bias(ap_dram, name):
        t = consts.tile([PDIM, 2], F32, name=name)
        nc.sync.dma_start(t[:], ap_dram.rearrange("(b p) -> p b", p=PDIM))
        return t

    bq_sb = load_bias(i_bq, "bq_sb")
    bk_sb = load_bias(i_bk, "bk_sb")
    bo_sb = load_bias(i_bo, "bo_sb")
    bvb_sb = consts.tile([PDIM, D], F32)
    nc.sync.dma_start(bvb_sb[:], i_bvb[:])
    kvb_sb = consts.tile([PDIM, NJ], F32)
    nc.sync.dma_start(kvb_sb[:], i_kvb.rearrange("(j p) -> p j", p=PDIM))

    acts = ctx.enter_context(tc.tile_pool(name="acts", bufs=1))
    QT = [acts.tile([PDIM, S], F32R, name=f"QT{b}") for b in range(2)]
    KT = [acts.tile([PDIM, SKV], F32R, name=f"KT{b}") for b in range(2)]
    # V in natural layout interleaved per head:
    # VNE[j] cols [h*64 : (h+1)*64] = [ones(32) | V_h(32)]
    VNE = [acts.tile([PDIM, 512], F32R, name=f"VNE{j}") for j in range(NJ)]
    # O.T as eight 32-row tiles so every normalize AP is at base partition 0
    OT = [acts.tile([32, S], F32R, name=f"OT{h}") for h in range(H)]
    wo8 = [acts.tile([32, D], F32R, name=f"wo8_{h}") for h in range(H)]
    for h in range(H):
        nc.sync.dma_start(wo8[h][:], i_wo[h * DH:(h + 1) * DH, :])

    # ---- projections ----
    with tc.tile_pool(name="proj_in", bufs=1) as proj_in, \
         tc.tile_pool(name="proj_ps", bufs=2, space="PSUM") as proj_ps:
        qt_sb = load2(i_qt, S)
        kt_sb = load2(i_kt, SKV)
        vt_sb = load2(i_vt, SKV)

        for dst, w_sb, b_sb, x_sb, ncols in (
            (QT, wq_sb, bq_sb, qt_sb, S),
            (KT, wk_sb, bk_sb, kt_sb, SKV),
        ):
            for ob in range(2):
                for c0, w in chunks(ncols):
                    ps = proj_ps.tile([PDIM, 512], F32, tag="projps")
                    for ib in range(2):
                        nc.tensor.matmul(
                            ps[:, 0:w],
                            lhsT=w_sb[ib][:, ob * PDIM:(ob + 1) * PDIM],
                            rhs=x_sb[ib][:, c0:c0 + w],
                            start=(ib == 0), stop=(ib == 1),
                        )
                    nc.vector.tensor_scalar_add(
                        dst[ob][:, c0:c0 + w], ps[:, 0:w], b_sb[:, ob:ob + 1])

        for j in range(NJ):
            ps = proj_ps.tile([PDIM, D], F32, tag="projps")
            for ib in range(2):
                nc.tensor.matmul(
                    ps[:],
                    lhsT=vt_sb[ib][:, j * PDIM:(j + 1) * PDIM],
                    rhs=wv_sb[ib][:],
                    start=(ib == 0), stop=(ib == 1),
                )
            # ones into the leading 32-col block per head, V+bias after
            v3 = VNE[j][:].rearrange("p (h c) -> p h c", c=2 * DH)
            nc.sync.dma_start(
                v3[:, :, 0:DH],
                i_vne1.rearrange("p (h c) -> p h c", c=DH))
            nc.vector.tensor_add(
                v3[:, :, DH:2 * DH],
                ps[:].rearrange("p (h c) -> p h c", c=DH),
                bvb_sb[:].rearrange("p (h c) -> p h c", c=DH))

    # ---- attention ----
    with tc.tile_pool(name="lps", bufs=2, space="PSUM") as lps_pool, \
         tc.tile_pool(name="pvps", bufs=2, space="PSUM") as pv_pool, \
         tc.tile_pool(name="psb", bufs=3) as p_pool, \
         tc.tile_pool(name="norm", bufs=2) as norm_pool:
        for ic in range(NQC):
            i0 = ic * 512
            for hp in range(4):
                h0, h1 = 2 * hp, 2 * hp + 1
                pva = pv_pool.tile([64, 512], F32, tag="pva")
                pvb = pv_pool.tile([64, 512], F32, tag="pvb")
                for j in range(NJ):
                    lt = lps_pool.tile([PDIM, 1024], F32, tag="lt")
                    for hh, h in enumerate((h0, h1)):
                        t = h // 4
                        bp = 32 * (h % 4)
                        nc.tensor.matmul(
                            lt[:, hh * 512:(hh + 1) * 512],
                            lhsT=KT[t][bp:bp + 32, j * PDIM:(j + 1) * PDIM],
                            rhs=QT[t][bp:bp + 32, i0:i0 + 512],
                            start=True, stop=True,
                            tile_position=(bp, 0),
                        )
                    pt = p_pool.tile([PDIM, 1024], F32R, tag="pt")
                    nc.scalar.activation(
                        pt[:], lt[:], mybir.ActivationFunctionType.Exp,
                        bias=kvb_sb[:, j:j + 1], scale=scale)
                    # fused PV + denominator per head, each into its own
                    # 1-bank PSUM tile: rows [den_h | num_h] at base 0
                    for pvt, h in ((pva, h0), (pvb, h1)):
                        nc.tensor.matmul(
                            pvt[:],
                            lhsT=VNE[j][:, h * 2 * DH:(h + 1) * 2 * DH],
                            rhs=pt[:, (h - h0) * 512:(h - h0 + 1) * 512],
                            start=(j == 0), stop=(j == NJ - 1),
                            tile_position=(0, 0),
                            skip_group_check=True,
                        )
                # normalize: pv rows = [den_h | num_h], all SBUF APs base 0
                for pvt, h in ((pva, h0), (pvb, h1)):
                    rec = norm_pool.tile([32, 512], F32, tag="rec")
                    nc.vector.reciprocal_approx_fast(rec[:], pvt[0:32, :])
                    nc.vector.tensor_mul(
                        OT[h][:, i0:i0 + 512], pvt[32:64, :], rec[:])

    # ---- output projection ----
    with tc.tile_pool(name="ops", bufs=2, space="PSUM") as out_ps, \
         tc.tile_pool(name="osb", bufs=2) as out_sb:
        for ob in range(2):
            for c in range(NQC):
                ps = out_ps.tile([PDIM, 512], F32, tag="ops")
                for h in range(H):
                    nc.tensor.matmul(
                        ps[:],
                        lhsT=wo8[h][:, ob * PDIM:(ob + 1) * PDIM],
                        rhs=OT[h][:, c * 512:(c + 1) * 512],
                        start=(h == 0), stop=(h == H - 1),
                    )
                ft = out_sb.tile([PDIM, 512], F32, tag="ft")
                nc.vector.tensor_scalar_add(ft[:], ps[:], bo_sb[:, ob:ob + 1])
                nc.sync.dma_start(
                    o_ot[ob * PDIM:(ob + 1) * PDIM, c * 512:(c + 1) * 512],
                    ft[:])


_PROGRAM_CACHE = {}

# DRAM dtypes: matmul operands land as float32r (same 4-byte payload; the
# PE rounds internally), everything else float32
_F32R_INPUTS = {"qt", "kt", "vt", "wq", "wk", "wv", "wo", "vne1"}


def _make_program(SKV, S=S):
    nc = bacc.Bacc("TRN2", target_bir_lowering=False, debug=False,
                   enable_asserts=False, num_devices=1)
    shapes = dict(qt=(D, S), kt=(D, SKV), vt=(D, SKV), wq=(D, D), wk=(D, D),
                  wv=(D, D), wo=(D, D), bq=(D,), bk=(D,), bo=(D,),
                  bvb=(PDIM, D), kvb=(SKV,), vne1=(PDIM, D))
    in_aps = [nc.dram_tensor(k, shapes[k],
                             F32R if k in _F32R_INPUTS else F32,
                             kind="ExternalInput").ap()
              for k in IN_NAMES]
    out_ap = nc.dram_tensor("ot", (D, S), F32, kind="ExternalOutput").ap()
    with tile.TileContext(nc) as tc:
        _mha_kernel(tc, [out_ap], in_aps, SKV=SKV, S=S)
    nc.compile()
    return nc


def _get_program(SKV):
    if SKV not in _PROGRAM_CACHE:
        _PROGRAM_CACHE[SKV] = _make_program(SKV)
    return _PROGRAM_CACHE[SKV]


def _prepare_in_maps(q, k, v, m, wq, bq, wk, bk, wv, bv, wo, bo):
    mask = np.asarray(m, np.float32).reshape(-1)
    keep = np.flatnonzero(mask == 0.0)
    skv = len(keep)
    assert skv > 0, "all kv positions masked"
    SKV = max(PDIM, ((skv + PDIM - 1) // PDIM) * PDIM)

    kvb = np.zeros(SKV, np.float32)
    kvb[skv:] = -1e9
    bvb = np.ascontiguousarray(np.tile(np.asarray(bv, np.float32)[None, :],
                                       (PDIM, 1)))
    common = dict(
        wq=np.ascontiguousarray(wq, np.float32),
        wk=np.ascontiguousarray(wk, np.float32),
        wv=np.ascontiguousarray(wv, np.float32),
        wo=np.ascontiguousarray(wo, np.float32),
        bq=np.ascontiguousarray(bq, np.float32),
        bk=np.ascontiguousarray(bk, np.float32),
        bo=np.ascontiguousarray(bo, np.float32),
        bvb=bvb, kvb=kvb,
        vne1=np.ones((PDIM, D), np.float32),
    )
    in_maps = []
    for b in range(B):
        kg = np.zeros((D, SKV), np.float32)
        vg = np.zeros((D, SKV), np.float32)
        kg[:, :skv] = np.asarray(k[b], np.float32).T[:, keep]
        vg[:, :skv] = np.asarray(v[b], np.float32).T[:, keep]
        in_maps.append(dict(
            qt=np.ascontiguousarray(np.asarray(q[b], np.float32).T),
            kt=kg, vt=vg, **common))
    return in_maps, SKV


def _run(q, k, v, m, wq, bq, wk, bk, wv, bv, wo, bo, trace=False):
    in_maps, SKV = _prepare_in_maps(q, k, v, m, wq, bq, wk, bk, wv, bv, wo, bo)
    nc = _get_program(SKV)
    res = bass_utils.run_bass_kernel_spmd(
        nc, in_maps, core_ids=list(range(N_CORES)), trace=trace)
    out = np.stack([res.results[b]["ot"].T for b in range(B)], axis=0)
    return np.ascontiguousarray(out, np.float32), res


def kernel(q, k, v, m, wq, bq, wk, bk, wv, bv, wo, bo):
    out, _ = _run(q, k, v, m, wq, bq, wk, bk, wv, bv, wo, bo, trace=False)
    return out
